# revision 13
# baseline (speedup 1.0000x reference)
"""Trainium2 Bass kernel for nn_NodeModel (GNN message passing).

Pipeline (per the reference):
  edge MLP:  h = elu(BN1(cat([x[row], edge_attr]) @ w1.T + b1))
  scatter-mean over destination nodes (col)
  node MLP:  out = elu(BN2(cat([x, agg]) @ w2.T + b2))

Sharding: edges are partitioned by destination-node shard (8 cores x 6250
nodes).  Within a core, edges are grouped into 64-node "bins" and padded to
128-edge blocks so every scatter matmul has a static PSUM target; the
one-hot scatter matrices are precomputed on the host.  BatchNorm biases b1/b2
cancel inside BN (training mode) and are dropped.  ELU is computed as
  elu(t) = t + relu(-t) + exp(min(t,0)) - 1
where the -1 after the edge MLP folds into BN2's mean and is only applied
explicitly at the final output.  The two BN statistics reductions are tiny
[128,2] AllReduces across the 8 cores.
"""

import sys

sys.path.insert(0, "/opt/trn_rl_repo")

from contextlib import ExitStack

import ml_dtypes
import numpy as np

import concourse.bass as bass
import concourse.bacc as bacc
import concourse.mybir as mybir
import concourse.tile as tile
from concourse.bass_utils import run_bass_kernel_spmd
from concourse.masks import make_identity

C = 128          # channels
N_NODES = 50000
N_EDGES = 800000
NCORES = 8
NS = N_NODES // NCORES   # nodes per core (6250)
BIN = 64         # node-bin width (one-hot span per block)
NBINS = (NS + BIN - 1) // BIN      # 98
WIN = 512        # PSUM scatter window width (8 bins)
NWIN = (NS + WIN - 1) // WIN       # 13
BLK = 128        # edges per scatter block (matmul K)
BPT = 4          # blocks per elementwise tile
TILE = BLK * BPT           # 512
EPS = 1e-5
ST2_CH = 250     # node-stats chunk (6250 = 25*250, equal-count bn_aggr)

F32 = mybir.dt.float32
BF16 = mybir.dt.bfloat16
npbf16 = ml_dtypes.bfloat16

_prog_cache = {}


# --------------------------------------------------------------------------
# host-side sharding / packing
# --------------------------------------------------------------------------

def _host_prep(x, edge_index, edge_attr, w1, w2, bn1_w, bn1_b, bn2_w, bn2_b):
    x32 = np.asarray(x, np.float32)
    ea32 = np.asarray(edge_attr, np.float32)
    row = np.asarray(edge_index[0], np.int64)
    col = np.asarray(edge_index[1], np.int64)
    E = row.shape[0]

    core = col // NS
    cloc = col % NS
    # stable sort by (core, local col)
    order = np.argsort(core * NS + cloc, kind="stable")
    row_s = row[order]
    cloc_s = cloc[order]
    core_s = core[order]
    binq_s = cloc_s // BIN

    # blocks per (core, bin); pad to max over cores so the program is uniform
    cb_s = core_s * NBINS + binq_s
    cnt_cb = np.bincount(cb_s, minlength=NCORES * NBINS).reshape(NCORES, NBINS)
    NB_q = np.maximum((cnt_cb + BLK - 1) // BLK, 1).max(axis=0)
    if NB_q.sum() % BPT:
        NB_q[-1] += BPT - NB_q.sum() % BPT
    NB = int(NB_q.sum())
    EPAD = NB * BLK
    blk_off_q = np.zeros(NBINS + 1, np.int64)
    np.cumsum(NB_q, out=blk_off_q[1:])

    # destination slot of each edge inside its core's padded stream
    grp_start = np.zeros(NCORES * NBINS + 1, np.int64)
    np.cumsum(cnt_cb.reshape(-1), out=grp_start[1:])
    pos_in_grp = np.arange(E, dtype=np.int64) - grp_start[cb_s]
    dest = blk_off_q[binq_s] * BLK + pos_in_grp

    T = NB // BPT
    in_maps = []
    w1T = np.ascontiguousarray(np.asarray(w1, np.float32).T)   # [2C, C]
    w2T = np.ascontiguousarray(np.asarray(w2, np.float32).T)
    for c in range(NCORES):
        m = core_s == c
        d = dest[m]
        xg = np.zeros((EPAD, C), np.float32)
        xg[d] = x32[row_s[m]]
        eas = np.zeros((EPAD, C), np.float32)
        eas[d] = ea32[order[m]]
        oh = np.zeros((EPAD, BIN), np.float32)
        oh[d, cloc_s[m] - binq_s[m] * BIN] = 1.0
        ohdev = (
            oh.reshape(T, BPT, BLK, BIN)
            .transpose(0, 2, 1, 3)
            .reshape(T, BLK, BPT * BIN)
        )
        # pair consecutive tiles: [T//2, BLK, 2*BPT*BIN]
        ohdev = (
            ohdev.reshape(T // 2, 2, BLK, BPT * BIN)
            .transpose(0, 2, 1, 3)
            .reshape(T // 2, BLK, 2 * BPT * BIN)
        )
        # merged [xg | ea] per 512-edge tile: xe[:, 1024t:1024t+512]=xg,
        # [+512:+1024]=ea
        xgT3 = np.ascontiguousarray(xg.T).reshape(C, T, TILE)
        eaT3 = np.ascontiguousarray(eas.T).reshape(C, T, TILE)
        xe = np.concatenate([xgT3, eaT3], axis=2).reshape(C, 2 * EPAD)
        cnt = np.bincount(cloc_s[m], minlength=NS).astype(np.float32)
        invc = (1.0 / np.maximum(cnt, 1.0)).astype(np.float32)[None, :]
        in_maps.append(
            {
                "xe": xe.astype(npbf16),
                "oh": ohdev.astype(npbf16),
                "invc": invc,
                "xTs": np.ascontiguousarray(x32[c * NS:(c + 1) * NS].T).astype(npbf16),
                "w1xT": np.ascontiguousarray(w1T[:C]).astype(npbf16),
                "w1eT": np.ascontiguousarray(w1T[C:]).astype(npbf16),
                "w2xT": np.ascontiguousarray(w2T[:C]).astype(npbf16),
                "w2aT": np.ascontiguousarray(w2T[C:]).astype(npbf16),
                "g1": np.asarray(bn1_w, np.float32).reshape(C, 1).copy(),
                "be1": np.asarray(bn1_b, np.float32).reshape(C, 1).copy(),
                "g2": np.asarray(bn2_w, np.float32).reshape(C, 1).copy(),
                "be2": np.asarray(bn2_b, np.float32).reshape(C, 1).copy(),
            }
        )
    meta = {"NB_q": tuple(int(v) for v in NB_q), "NB": NB, "EPAD": EPAD, "T": T}
    return in_maps, meta


# --------------------------------------------------------------------------
# device program
# --------------------------------------------------------------------------

def _bn_coeffs(nc, pool, var, mu, gamma, beta):
    """a = gamma*rsqrt(var+eps);  bb = beta - mu*a  (all [C,1] fp32)."""
    AL = mybir.AluOpType
    t = pool.tile([C, 1], F32, tag="bnc_t")
    nc.vector.tensor_scalar(t[:], var[:], EPS, None, AL.add)
    r = pool.tile([C, 1], F32, tag="bnc_r")
    nc.vector.reciprocal(r[:], t[:])                    # 1/(var+eps)
    s = pool.tile([C, 1], F32, tag="bnc_s")
    nc.scalar.sqrt(s[:], r[:])                          # ~rsqrt(var+eps)
    # one Newton step: s <- 0.5*s*(3 - t*s^2)
    u = pool.tile([C, 1], F32, tag="bnc_u")
    nc.vector.tensor_tensor(u[:], s[:], s[:], op=AL.mult)
    nc.vector.tensor_tensor(u[:], u[:], t[:], op=AL.mult)
    nc.vector.tensor_scalar(u[:], u[:], -1.0, 3.0, AL.mult, AL.add)
    nc.vector.tensor_tensor(s[:], s[:], u[:], op=AL.mult)
    a = pool.tile([C, 1], F32, tag="bnc_a")
    nc.vector.tensor_scalar(a[:], s[:], 0.5, None, AL.mult)
    nc.vector.tensor_tensor(a[:], a[:], gamma[:], op=AL.mult)
    bb = pool.tile([C, 1], F32, tag="bnc_bb")
    nc.vector.tensor_tensor(bb[:], mu[:], a[:], op=AL.mult)
    nc.vector.tensor_tensor(bb[:], beta[:], bb[:], op=AL.subtract)
    return a, bb


def _build_program(NB_q):
    AL = mybir.AluOpType
    AF = mybir.ActivationFunctionType
    NB_q = list(NB_q)
    NB = sum(NB_q)
    EPAD = NB * BLK
    T = NB // BPT

    # block -> bin, and first/last block per bin / per window
    blk_bin = []
    for q, nbq in enumerate(NB_q):
        blk_bin += [q] * nbq
    bin_first, bin_last = {}, {}
    for b, q in enumerate(blk_bin):
        bin_first.setdefault(q, b)
        bin_last[q] = b
    win_first, win_last = {}, {}
    for b, q in enumerate(blk_bin):
        w = q // 8
        win_first.setdefault(w, b)
        win_last[w] = b

    nc = bacc.Bacc(None, num_devices=NCORES)
    xe_d = nc.dram_tensor("xe", [C, 2 * EPAD], BF16, kind="ExternalInput")
    oh_d = nc.dram_tensor("oh", [T // 2, BLK, 2 * BPT * BIN], BF16,
                          kind="ExternalInput")
    invc_d = nc.dram_tensor("invc", [1, NS], F32, kind="ExternalInput")
    xTs_d = nc.dram_tensor("xTs", [C, NS], BF16, kind="ExternalInput")
    w1x_d = nc.dram_tensor("w1xT", [C, C], BF16, kind="ExternalInput")
    w1e_d = nc.dram_tensor("w1eT", [C, C], BF16, kind="ExternalInput")
    w2x_d = nc.dram_tensor("w2xT", [C, C], BF16, kind="ExternalInput")
    w2a_d = nc.dram_tensor("w2aT", [C, C], BF16, kind="ExternalInput")
    g1_d = nc.dram_tensor("g1", [C, 1], F32, kind="ExternalInput")
    be1_d = nc.dram_tensor("be1", [C, 1], F32, kind="ExternalInput")
    g2_d = nc.dram_tensor("g2", [C, 1], F32, kind="ExternalInput")
    be2_d = nc.dram_tensor("be2", [C, 1], F32, kind="ExternalInput")
    out_d = nc.dram_tensor("out", [NS, C], F32, kind="ExternalOutput")

    h_d = nc.dram_tensor("hbuf", [C, EPAD], BF16)
    cc1i_d = nc.dram_tensor("cc1i", [C, 2], F32)
    cc1o_d = nc.dram_tensor("cc1o", [C, 2], F32, addr_space="Shared")
    cc2i_d = nc.dram_tensor("cc2i", [C, 2], F32)
    cc2o_d = nc.dram_tensor("cc2o", [C, 2], F32, addr_space="Shared")

    rg = [list(range(NCORES))]

    with ExitStack() as ctx:
        tc = ctx.enter_context(tile.TileContext(nc))
        const = ctx.enter_context(tc.tile_pool(name="const", bufs=1))
        io_p = ctx.enter_context(tc.tile_pool(name="io", bufs=6))
        mid_p = ctx.enter_context(tc.tile_pool(name="mid", bufs=4))
        st_p = ctx.enter_context(tc.tile_pool(name="st", bufs=1))
        ps_h = ctx.enter_context(tc.tile_pool(name="psh", bufs=4, space="PSUM"))
        ps_s = ctx.enter_context(tc.tile_pool(name="pss", bufs=2, space="PSUM"))
        ps_w = ctx.enter_context(tc.tile_pool(name="psw", bufs=2, space="PSUM"))

        ident = const.tile([128, 128], F32)
        make_identity(nc, ident[:])
        w1x_sb = const.tile([C, C], BF16, tag="w1x")
        nc.sync.dma_start(w1x_sb[:], w1x_d[:])
        w1e_sb = const.tile([C, C], BF16, tag="w1e")
        nc.sync.dma_start(w1e_sb[:], w1e_d[:])
        w2x_sb = const.tile([C, C], BF16, tag="w2x")
        nc.sync.dma_start(w2x_sb[:], w2x_d[:])
        w2a_sb = const.tile([C, C], BF16, tag="w2a")
        nc.sync.dma_start(w2a_sb[:], w2a_d[:])
        g1_sb = const.tile([C, 1], F32, tag="g1")
        nc.sync.dma_start(g1_sb[:], g1_d[:])
        be1_sb = const.tile([C, 1], F32, tag="be1")
        nc.sync.dma_start(be1_sb[:], be1_d[:])
        g2_sb = const.tile([C, 1], F32, tag="g2")
        nc.sync.dma_start(g2_sb[:], g2_d[:])
        be2_sb = const.tile([C, 1], F32, tag="be2")
        nc.sync.dma_start(be2_sb[:], be2_d[:])
        invc_sb = const.tile([C, NS], F32, tag="invc")
        nc.gpsimd.dma_start(invc_sb[:], invc_d[:].to_broadcast([C, NS]))

        # ---------------- pass 1: edge MLP -> h (bf16, ch-major), stats ----
        # process tiles in pairs so each weight matrix loads once per pair
        stats1 = st_p.tile([C, 6 * T], F32, tag="stats1")
        for tp_i in range(T // 2):
            xe_t = io_p.tile([C, 2 * TILE], BF16, tag="xe0")
            nc.sync.dma_start(xe_t[:], xe_d[:, tp_i * 2048:tp_i * 2048 + 1024])
            xe_u = io_p.tile([C, 2 * TILE], BF16, tag="xe1")
            nc.sync.dma_start(xe_u[:], xe_d[:, tp_i * 2048 + 1024:(tp_i + 1) * 2048])
            pha = ps_h.tile([C, TILE], F32, tag="ph", name=f"pha{tp_i}")
            phb = ps_h.tile([C, TILE], F32, tag="ph", name=f"phb{tp_i}")
            nc.tensor.matmul(pha[:], w1x_sb[:], xe_t[:, :TILE], start=True, stop=False)
            nc.tensor.matmul(phb[:], w1x_sb[:], xe_u[:, :TILE], start=True, stop=False)
            nc.tensor.matmul(pha[:], w1e_sb[:], xe_t[:, TILE:], start=False, stop=True)
            nc.tensor.matmul(phb[:], w1e_sb[:], xe_u[:, TILE:], start=False, stop=True)
            h_sb = mid_p.tile([C, 2 * TILE], BF16, tag="h1")
            nc.scalar.copy(h_sb[:, :TILE], pha[:])
            nc.scalar.copy(h_sb[:, TILE:], phb[:])
            t2 = 2 * tp_i
            nc.vector.bn_stats(stats1[:, 6 * t2:6 * t2 + 6], h_sb[:, :TILE])
            nc.vector.bn_stats(stats1[:, 6 * t2 + 6:6 * t2 + 12], h_sb[:, TILE:])
            nc.sync.dma_start(h_d[:, t2 * TILE:(t2 + 2) * TILE], h_sb[:])

        # ---------------- BN1 stats: aggregate + AllReduce ------------------
        mv1 = st_p.tile([C, 2], F32, tag="mv1")
        nc.vector.bn_aggr(mv1[:], stats1[:])
        cc1_sb = st_p.tile([C, 2], F32, tag="cc1s")
        nc.vector.tensor_copy(cc1_sb[:, 0:1], mv1[:, 0:1])
        nc.vector.tensor_tensor(cc1_sb[:, 1:2], mv1[:, 0:1], mv1[:, 0:1], op=AL.mult)
        nc.vector.tensor_tensor(cc1_sb[:, 1:2], cc1_sb[:, 1:2], mv1[:, 1:2], op=AL.add)
        nc.sync.dma_start(cc1i_d[:], cc1_sb[:])
        nc.gpsimd.collective_compute(
            "AllReduce", AL.add, replica_groups=rg, ins=[cc1i_d[:]], outs=[cc1o_d[:]]
        )
        cc1r = st_p.tile([C, 2], F32, tag="cc1r")
        nc.sync.dma_start(cc1r[:], cc1o_d[:])
        ratio1 = float(EPAD) / float(N_EDGES)   # rescale for zero pads
        mu1 = st_p.tile([C, 1], F32, tag="mu1")
        nc.vector.tensor_scalar(mu1[:], cc1r[:, 0:1], ratio1, None, AL.mult)
        var1 = st_p.tile([C, 1], F32, tag="var1")
        nc.vector.tensor_tensor(var1[:], mu1[:], mu1[:], op=AL.mult)
        nc.vector.tensor_scalar(var1[:], var1[:], -1.0, None, AL.mult)
        ex1 = st_p.tile([C, 1], F32, tag="ex1")
        nc.vector.tensor_scalar(ex1[:], cc1r[:, 1:2], ratio1, None, AL.mult)
        nc.vector.tensor_tensor(var1[:], var1[:], ex1[:], op=AL.add)
        a1, bb1 = _bn_coeffs(nc, st_p, var1, mu1, g1_sb, be1_sb)

        # ---------------- pass 2: BN1+ELU, transpose, scatter ---------------
        # elu(t)+1 = t + relu(-t) + exp(min(t,0)); terms summed (gpsimd) in
        # ch-major bf16, then one PE transpose per 128-edge block.
        aggT = const.tile([C, NS], BF16, tag="aggT")
        pw = {}
        ident_bf = const.tile([128, 128], BF16, tag="identbf")
        nc.vector.tensor_copy(ident_bf[:], ident[:])
        for tp_i in range(T // 2):
            h_t = io_p.tile([C, 2 * TILE], BF16, tag="h2")
            nc.sync.dma_start(h_t[:], h_d[:, tp_i * 2 * TILE:(tp_i + 1) * 2 * TILE])
            oh_t = io_p.tile([BLK, 2 * BPT * BIN], BF16, tag="oh")
            nc.sync.dma_start(oh_t[:], oh_d[tp_i])
            for half in range(2):
                t = 2 * tp_i + half
                hsl = slice(half * TILE, (half + 1) * TILE)
                tp = mid_p.tile([C, TILE], BF16, tag="tp")
                nc.vector.tensor_scalar(tp[:], h_t[:, hsl], a1[:], bb1[:],
                                        AL.mult, AL.add)
                r2 = mid_p.tile([C, TILE], BF16, tag="r2")
                nc.scalar.activation(r2[:], tp[:], AF.Relu, scale=-1.0)
                ee = mid_p.tile([C, TILE], BF16, tag="ee")
                nc.scalar.activation(ee[:], r2[:], AF.Exp, scale=-1.0)
                ss = mid_p.tile([C, TILE], BF16, tag="ss")
                nc.gpsimd.tensor_tensor(ss[:], tp[:], r2[:], op=AL.add)
                nc.gpsimd.tensor_tensor(ss[:], ss[:], ee[:], op=AL.add)
                pS = ps_s.tile([128, TILE], BF16, tag="pS")
                for bi in range(BPT):
                    bsl = slice(bi * BLK, (bi + 1) * BLK)
                    nc.tensor.matmul(pS[:, bsl], ss[:, bsl], ident_bf[:],
                                     is_transpose=True, start=True, stop=True)
                s_sb = mid_p.tile([128, TILE], BF16, tag="s")
                nc.vector.tensor_copy(s_sb[:], pS[:])
                for bi in range(BPT):
                    b = t * BPT + bi
                    q = blk_bin[b]
                    w = q // 8
                    off = BIN * (q % 8)
                    if b == win_first[w]:
                        pw[w] = ps_w.tile([C, WIN], F32, tag="pw", name=f"pw{w}")
                    nc.tensor.matmul(
                        pw[w][:, off:off + BIN],
                        s_sb[:, bi * BLK:(bi + 1) * BLK],
                        oh_t[:, (half * BPT + bi) * BIN:(half * BPT + bi + 1) * BIN],
                        start=(b == bin_first[q]),
                        stop=(b == bin_last[q]),
                    )
                    if b == win_last[w]:
                        wb = w * WIN
                        ww = min(WIN, NS - wb)
                        nc.vector.tensor_tensor(
                            aggT[:, wb:wb + ww],
                            pw[w][:, :ww],
                            invc_sb[:, wb:wb + ww],
                            op=AL.mult,
                        )

        # ---------------- node MLP ------------------------------------------
        xT_sb = const.tile([C, NS], BF16, tag="xTs")
        nc.sync.dma_start(xT_sb[:], xTs_d[:])
        o2_sb = const.tile([C, NS], BF16, tag="o2")
        for w in range(NWIN):
            wb = w * WIN
            ww = min(WIN, NS - wb)
            p2 = ps_h.tile([C, TILE], F32, tag="ph")
            nc.tensor.matmul(p2[:, :ww], w2x_sb[:], xT_sb[:, wb:wb + ww],
                             start=True, stop=False)
            nc.tensor.matmul(p2[:, :ww], w2a_sb[:], aggT[:, wb:wb + ww],
                             start=False, stop=True)
            nc.scalar.copy(o2_sb[:, wb:wb + ww], p2[:, :ww])

        stats2 = st_p.tile([C, 6 * (NS // ST2_CH)], F32, tag="stats2")
        for k in range(NS // ST2_CH):
            nc.vector.bn_stats(stats2[:, 6 * k:6 * k + 6],
                               o2_sb[:, k * ST2_CH:(k + 1) * ST2_CH])
        mv2 = st_p.tile([C, 2], F32, tag="mv2")
        nc.vector.bn_aggr(mv2[:], stats2[:])
        cc2_sb = st_p.tile([C, 2], F32, tag="cc2s")
        nc.vector.tensor_copy(cc2_sb[:, 0:1], mv2[:, 0:1])
        nc.vector.tensor_tensor(cc2_sb[:, 1:2], mv2[:, 0:1], mv2[:, 0:1], op=AL.mult)
        nc.vector.tensor_tensor(cc2_sb[:, 1:2], cc2_sb[:, 1:2], mv2[:, 1:2], op=AL.add)
        nc.sync.dma_start(cc2i_d[:], cc2_sb[:])
        nc.gpsimd.collective_compute(
            "AllReduce", AL.add, replica_groups=rg, ins=[cc2i_d[:]], outs=[cc2o_d[:]]
        )
        cc2r = st_p.tile([C, 2], F32, tag="cc2r")
        nc.sync.dma_start(cc2r[:], cc2o_d[:])
        mu2 = st_p.tile([C, 1], F32, tag="mu2")
        nc.vector.tensor_scalar(mu2[:], cc2r[:, 0:1], 1.0 / NCORES, None, AL.mult)
        var2 = st_p.tile([C, 1], F32, tag="var2")
        nc.vector.tensor_tensor(var2[:], mu2[:], mu2[:], op=AL.mult)
        nc.vector.tensor_scalar(var2[:], var2[:], -1.0, None, AL.mult)
        ex2 = st_p.tile([C, 1], F32, tag="ex2")
        nc.vector.tensor_scalar(ex2[:], cc2r[:, 1:2], 1.0 / NCORES, None, AL.mult)
        nc.vector.tensor_tensor(var2[:], var2[:], ex2[:], op=AL.add)
        a2, bb2 = _bn_coeffs(nc, st_p, var2, mu2, g2_sb, be2_sb)

        # ---------------- final elementwise + transpose + store -------------
        nblocks = (NS + BLK - 1) // BLK
        for nb in range(nblocks):
            ob = nb * BLK
            ow = min(BLK, NS - ob)
            tp2 = mid_p.tile([C, BLK], BF16, tag="tp2")
            nc.vector.tensor_scalar(tp2[:, :ow], o2_sb[:, ob:ob + ow],
                                    a2[:], bb2[:], AL.mult, AL.add)
            r22 = mid_p.tile([C, BLK], BF16, tag="r22")
            nc.scalar.activation(r22[:, :ow], tp2[:, :ow], AF.Relu, scale=-1.0)
            e22 = mid_p.tile([C, BLK], BF16, tag="e22")
            nc.scalar.activation(e22[:, :ow], r22[:, :ow], AF.Exp, scale=-1.0)
            s2 = mid_p.tile([C, BLK], BF16, tag="s2")
            nc.gpsimd.tensor_tensor(s2[:, :ow], tp2[:, :ow], r22[:, :ow], op=AL.add)
            nc.gpsimd.tensor_tensor(s2[:, :ow], s2[:, :ow], e22[:, :ow], op=AL.add)
            pO = ps_s.tile([128, TILE], BF16, tag="pS")
            nc.tensor.matmul(pO[:ow, :C], s2[:, :ow], ident_bf[:],
                             is_transpose=True, start=True, stop=True)
            o_sb = mid_p.tile([128, C], F32, tag="osb")
            nc.vector.tensor_scalar(o_sb[:ow, :], pO[:ow, :C], 1.0, -1.0,
                                    AL.mult, AL.add)
            nc.sync.dma_start(out_d[ob:ob + ow, :], o_sb[:ow, :])

    nc.finalize()
    return nc


# --------------------------------------------------------------------------
# entry point
# --------------------------------------------------------------------------

def kernel(x, edge_index, edge_attr, u, batch,
           w1, b1, bn1_w, bn1_b, w2, b2, bn2_w, bn2_b,
           _trace=False):
    in_maps, meta = _host_prep(
        x, edge_index, edge_attr, w1, w2, bn1_w, bn1_b, bn2_w, bn2_b
    )
    key = meta["NB_q"]
    if key not in _prog_cache:
        _prog_cache[key] = _build_program(key)
    nc = _prog_cache[key]
    res = run_bass_kernel_spmd(nc, in_maps, list(range(NCORES)), trace=_trace)
    out = np.concatenate([r["out"] for r in res.results], axis=0)
    if _trace:
        kernel.last_results = res
    return out.astype(np.float32)


# revision 18
# speedup vs baseline: 1.2304x; 1.2304x over previous
"""Trainium2 Bass kernel for nn_NodeModel (GNN message passing).

Pipeline (per the reference):
  edge MLP:  h = elu(BN1(cat([x[row], edge_attr]) @ w1.T + b1))
  scatter-mean over destination nodes (col)
  node MLP:  out = elu(BN2(cat([x, agg]) @ w2.T + b2))

Sharding: edges are partitioned by destination-node shard (8 cores x 6250
nodes).  Within a core, edges are grouped into 64-node "bins" and padded to
128-edge blocks so every scatter matmul has a static PSUM target; the
one-hot scatter matrices are precomputed on the host.  BatchNorm biases b1/b2
cancel inside BN (training mode) and are dropped.  ELU is computed as
  elu(t) = t + relu(-t) + exp(min(t,0)) - 1
where the -1 after the edge MLP folds into BN2's mean and is only applied
explicitly at the final output.  The two BN statistics reductions are tiny
[128,2] AllReduces across the 8 cores.
"""

import sys

sys.path.insert(0, "/opt/trn_rl_repo")

from contextlib import ExitStack

import ml_dtypes
import numpy as np

import concourse.bass as bass
import concourse.bacc as bacc
import concourse.mybir as mybir
import concourse.tile as tile
from concourse.bass_utils import run_bass_kernel_spmd
from concourse.masks import make_identity

C = 128          # channels
N_NODES = 50000
N_EDGES = 800000
NCORES = 8
NS = N_NODES // NCORES   # nodes per core (6250)
BIN = 64         # node-bin width (one-hot span per block)
NBINS = (NS + BIN - 1) // BIN      # 98
WIN = 512        # PSUM scatter window width (8 bins)
NWIN = (NS + WIN - 1) // WIN       # 13
BLK = 128        # edges per scatter block (matmul K)
BPT = 4          # blocks per elementwise tile
TILE = BLK * BPT           # 512
EPS = 1e-5
ST2_CH = 250     # node-stats chunk (6250 = 25*250, equal-count bn_aggr)

F32 = mybir.dt.float32
BF16 = mybir.dt.bfloat16
npbf16 = ml_dtypes.bfloat16

_prog_cache = {}


# --------------------------------------------------------------------------
# host-side sharding / packing
# --------------------------------------------------------------------------

def _host_prep(x, edge_index, edge_attr, w1, w2, bn1_w, bn1_b, bn2_w, bn2_b):
    x32 = np.asarray(x, np.float32)
    ea32 = np.asarray(edge_attr, np.float32)
    row = np.asarray(edge_index[0], np.int64)
    col = np.asarray(edge_index[1], np.int64)
    E = row.shape[0]

    core = col // NS
    cloc = col % NS
    # stable sort by (core, local col)
    order = np.argsort(core * NS + cloc, kind="stable")
    row_s = row[order]
    cloc_s = cloc[order]
    core_s = core[order]
    binq_s = cloc_s // BIN

    # blocks per (core, bin); pad to max over cores so the program is uniform
    cb_s = core_s * NBINS + binq_s
    cnt_cb = np.bincount(cb_s, minlength=NCORES * NBINS).reshape(NCORES, NBINS)
    NB_q = np.maximum((cnt_cb + BLK - 1) // BLK, 1).max(axis=0)
    if NB_q.sum() % BPT:
        NB_q[-1] += BPT - NB_q.sum() % BPT
    NB = int(NB_q.sum())
    EPAD = NB * BLK
    blk_off_q = np.zeros(NBINS + 1, np.int64)
    np.cumsum(NB_q, out=blk_off_q[1:])

    # destination slot of each edge inside its core's padded stream
    grp_start = np.zeros(NCORES * NBINS + 1, np.int64)
    np.cumsum(cnt_cb.reshape(-1), out=grp_start[1:])
    pos_in_grp = np.arange(E, dtype=np.int64) - grp_start[cb_s]
    dest = blk_off_q[binq_s] * BLK + pos_in_grp

    T = NB // BPT
    in_maps = []
    w1T = np.ascontiguousarray(np.asarray(w1, np.float32).T)   # [2C, C]
    w2T = np.ascontiguousarray(np.asarray(w2, np.float32).T)
    for c in range(NCORES):
        m = core_s == c
        d = dest[m]
        xg = np.zeros((EPAD, C), np.float32)
        xg[d] = x32[row_s[m]]
        eas = np.zeros((EPAD, C), np.float32)
        eas[d] = ea32[order[m]]
        oh = np.zeros((EPAD, BIN), np.float32)
        oh[d, cloc_s[m] - binq_s[m] * BIN] = 1.0
        ohdev = (
            oh.reshape(T, BPT, BLK, BIN)
            .transpose(0, 2, 1, 3)
            .reshape(T, BLK, BPT * BIN)
        )
        # pair consecutive tiles: [T//2, BLK, 2*BPT*BIN]
        ohdev = (
            ohdev.reshape(T // 2, 2, BLK, BPT * BIN)
            .transpose(0, 2, 1, 3)
            .reshape(T // 2, BLK, 2 * BPT * BIN)
        )
        # merged [xg | ea] per 512-edge tile: xe[:, 1024t:1024t+512]=xg,
        # [+512:+1024]=ea
        xgT3 = np.ascontiguousarray(xg.T).reshape(C, T, TILE)
        eaT3 = np.ascontiguousarray(eas.T).reshape(C, T, TILE)
        xe = np.concatenate([xgT3, eaT3], axis=2).reshape(C, 2 * EPAD)
        cnt = np.bincount(cloc_s[m], minlength=NS).astype(np.float32)
        invc = (1.0 / np.maximum(cnt, 1.0)).astype(np.float32)[None, :]
        in_maps.append(
            {
                "xe": xe.astype(npbf16),
                "oh": ohdev.astype(npbf16),
                "invc": invc,
                "xTs": np.ascontiguousarray(x32[c * NS:(c + 1) * NS].T).astype(npbf16),
                "w1xT": np.ascontiguousarray(w1T[:C]).astype(npbf16),
                "w1eT": np.ascontiguousarray(w1T[C:]).astype(npbf16),
                "w2xT": np.ascontiguousarray(w2T[:C]).astype(npbf16),
                "w2aT": np.ascontiguousarray(w2T[C:]).astype(npbf16),
                "g1": np.asarray(bn1_w, np.float32).reshape(C, 1).copy(),
                "be1": np.asarray(bn1_b, np.float32).reshape(C, 1).copy(),
                "g2": np.asarray(bn2_w, np.float32).reshape(C, 1).copy(),
                "be2": np.asarray(bn2_b, np.float32).reshape(C, 1).copy(),
            }
        )
    meta = {"NB_q": tuple(int(v) for v in NB_q), "NB": NB, "EPAD": EPAD, "T": T}
    return in_maps, meta


# --------------------------------------------------------------------------
# device program
# --------------------------------------------------------------------------

def _bn_coeffs(nc, pool, var, mu, gamma, beta):
    """a = gamma*rsqrt(var+eps);  bb = beta - mu*a  (all [C,1] fp32)."""
    AL = mybir.AluOpType
    t = pool.tile([C, 1], F32, tag="bnc_t")
    nc.vector.tensor_scalar(t[:], var[:], EPS, None, AL.add)
    r = pool.tile([C, 1], F32, tag="bnc_r")
    nc.vector.reciprocal(r[:], t[:])                    # 1/(var+eps)
    s = pool.tile([C, 1], F32, tag="bnc_s")
    nc.scalar.sqrt(s[:], r[:])                          # ~rsqrt(var+eps)
    # one Newton step: s <- 0.5*s*(3 - t*s^2)
    u = pool.tile([C, 1], F32, tag="bnc_u")
    nc.vector.tensor_tensor(u[:], s[:], s[:], op=AL.mult)
    nc.vector.tensor_tensor(u[:], u[:], t[:], op=AL.mult)
    nc.vector.tensor_scalar(u[:], u[:], -1.0, 3.0, AL.mult, AL.add)
    nc.vector.tensor_tensor(s[:], s[:], u[:], op=AL.mult)
    a = pool.tile([C, 1], F32, tag="bnc_a")
    nc.vector.tensor_scalar(a[:], s[:], 0.5, None, AL.mult)
    nc.vector.tensor_tensor(a[:], a[:], gamma[:], op=AL.mult)
    bb = pool.tile([C, 1], F32, tag="bnc_bb")
    nc.vector.tensor_tensor(bb[:], mu[:], a[:], op=AL.mult)
    nc.vector.tensor_tensor(bb[:], beta[:], bb[:], op=AL.subtract)
    bbp1 = pool.tile([C, 1], F32, tag="bnc_bbp1")
    nc.vector.tensor_scalar(bbp1[:], bb[:], 1.0, None, AL.add)
    return a, bb, bbp1


def _build_program(NB_q):
    AL = mybir.AluOpType
    AF = mybir.ActivationFunctionType
    NB_q = list(NB_q)
    NB = sum(NB_q)
    EPAD = NB * BLK
    T = NB // BPT

    # block -> bin, and first/last block per bin / per window
    blk_bin = []
    for q, nbq in enumerate(NB_q):
        blk_bin += [q] * nbq
    bin_first, bin_last = {}, {}
    for b, q in enumerate(blk_bin):
        bin_first.setdefault(q, b)
        bin_last[q] = b
    win_first, win_last = {}, {}
    for b, q in enumerate(blk_bin):
        w = q // 8
        win_first.setdefault(w, b)
        win_last[w] = b

    nc = bacc.Bacc(None, num_devices=NCORES)
    xe_d = nc.dram_tensor("xe", [C, 2 * EPAD], BF16, kind="ExternalInput")
    oh_d = nc.dram_tensor("oh", [T // 2, BLK, 2 * BPT * BIN], BF16,
                          kind="ExternalInput")
    invc_d = nc.dram_tensor("invc", [1, NS], F32, kind="ExternalInput")
    xTs_d = nc.dram_tensor("xTs", [C, NS], BF16, kind="ExternalInput")
    w1x_d = nc.dram_tensor("w1xT", [C, C], BF16, kind="ExternalInput")
    w1e_d = nc.dram_tensor("w1eT", [C, C], BF16, kind="ExternalInput")
    w2x_d = nc.dram_tensor("w2xT", [C, C], BF16, kind="ExternalInput")
    w2a_d = nc.dram_tensor("w2aT", [C, C], BF16, kind="ExternalInput")
    g1_d = nc.dram_tensor("g1", [C, 1], F32, kind="ExternalInput")
    be1_d = nc.dram_tensor("be1", [C, 1], F32, kind="ExternalInput")
    g2_d = nc.dram_tensor("g2", [C, 1], F32, kind="ExternalInput")
    be2_d = nc.dram_tensor("be2", [C, 1], F32, kind="ExternalInput")
    out_d = nc.dram_tensor("out", [NS, C], F32, kind="ExternalOutput")

    h_d = nc.dram_tensor("hbuf", [C, EPAD], BF16)
    cc1i_d = nc.dram_tensor("cc1i", [C, 2], F32)
    cc1o_d = nc.dram_tensor("cc1o", [C, 2], F32, addr_space="Shared")
    cc2i_d = nc.dram_tensor("cc2i", [C, 2], F32)
    cc2o_d = nc.dram_tensor("cc2o", [C, 2], F32, addr_space="Shared")

    rg = [list(range(NCORES))]

    with ExitStack() as ctx:
        tc = ctx.enter_context(tile.TileContext(nc))
        const = ctx.enter_context(tc.tile_pool(name="const", bufs=1))
        io_p = ctx.enter_context(tc.tile_pool(name="io", bufs=6))
        mid_p = ctx.enter_context(tc.tile_pool(name="mid", bufs=4))
        st_p = ctx.enter_context(tc.tile_pool(name="st", bufs=1))
        ps_h = ctx.enter_context(tc.tile_pool(name="psh", bufs=4, space="PSUM"))
        ps_s = ctx.enter_context(tc.tile_pool(name="pss", bufs=2, space="PSUM"))
        ps_w = ctx.enter_context(tc.tile_pool(name="psw", bufs=2, space="PSUM"))

        ident = const.tile([128, 128], F32)
        make_identity(nc, ident[:])
        w1x_sb = const.tile([C, C], BF16, tag="w1x")
        nc.sync.dma_start(w1x_sb[:], w1x_d[:])
        w1e_sb = const.tile([C, C], BF16, tag="w1e")
        nc.sync.dma_start(w1e_sb[:], w1e_d[:])
        w2x_sb = const.tile([C, C], BF16, tag="w2x")
        nc.sync.dma_start(w2x_sb[:], w2x_d[:])
        w2a_sb = const.tile([C, C], BF16, tag="w2a")
        nc.sync.dma_start(w2a_sb[:], w2a_d[:])
        g1_sb = const.tile([C, 1], F32, tag="g1")
        nc.sync.dma_start(g1_sb[:], g1_d[:])
        be1_sb = const.tile([C, 1], F32, tag="be1")
        nc.sync.dma_start(be1_sb[:], be1_d[:])
        g2_sb = const.tile([C, 1], F32, tag="g2")
        nc.sync.dma_start(g2_sb[:], g2_d[:])
        be2_sb = const.tile([C, 1], F32, tag="be2")
        nc.sync.dma_start(be2_sb[:], be2_d[:])
        invc_sb = const.tile([C, NS], F32, tag="invc")
        nc.gpsimd.dma_start(invc_sb[:], invc_d[:].to_broadcast([C, NS]))

        # ---------------- pass 1: edge MLP -> h (bf16, ch-major), stats ----
        # tiles processed in pairs (one weight load per matrix per pair);
        # stats split across engines: half A uses ACT copy+accum_out (sum) +
        # ACT Square+accum_out (sumsq); half B uses DVE copy + bn_stats.
        P2N = T // 2
        sumA = st_p.tile([C, P2N], F32, tag="sumA")
        ssqA = st_p.tile([C, P2N], F32, tag="ssqA")
        statsB = st_p.tile([C, 6 * P2N], F32, tag="statsB")
        for tp_i in range(P2N):
            xe_t = io_p.tile([C, 2 * TILE], BF16, tag="xe0")
            nc.sync.dma_start(xe_t[:], xe_d[:, tp_i * 2048:tp_i * 2048 + 1024])
            xe_u = io_p.tile([C, 2 * TILE], BF16, tag="xe1")
            nc.sync.dma_start(xe_u[:], xe_d[:, tp_i * 2048 + 1024:(tp_i + 1) * 2048])
            pha = ps_h.tile([C, TILE], F32, tag="ph", name=f"pha{tp_i}")
            phb = ps_h.tile([C, TILE], F32, tag="ph", name=f"phb{tp_i}")
            nc.tensor.matmul(pha[:], w1x_sb[:], xe_t[:, :TILE], start=True, stop=False)
            nc.tensor.matmul(phb[:], w1x_sb[:], xe_u[:, :TILE], start=True, stop=False)
            nc.tensor.matmul(pha[:], w1e_sb[:], xe_t[:, TILE:], start=False, stop=True)
            nc.tensor.matmul(phb[:], w1e_sb[:], xe_u[:, TILE:], start=False, stop=True)
            h_sb = mid_p.tile([C, 2 * TILE], BF16, tag="h1")
            nc.scalar.activation(h_sb[:, :TILE], pha[:], AF.Copy,
                                 accum_out=sumA[:, tp_i:tp_i + 1])
            sq_scr = mid_p.tile([C, TILE], BF16, tag="sqscr")
            nc.scalar.activation(sq_scr[:], h_sb[:, :TILE], AF.Square,
                                 accum_out=ssqA[:, tp_i:tp_i + 1])
            nc.vector.tensor_copy(h_sb[:, TILE:], phb[:])
            nc.vector.bn_stats(statsB[:, 6 * tp_i:6 * tp_i + 6], h_sb[:, TILE:])
            nc.sync.dma_start(h_d[:, tp_i * 2 * TILE:(tp_i + 1) * 2 * TILE], h_sb[:])

        # ---------------- BN1 stats: combine + AllReduce --------------------
        # raw sums: S = sum(sumA) + meanB*(EPAD/2); SS = sum(ssqA) +
        # (varB+meanB^2)*(EPAD/2)
        mv1 = st_p.tile([C, 2], F32, tag="mv1")
        nc.vector.bn_aggr(mv1[:], statsB[:])
        cc1_sb = st_p.tile([C, 2], F32, tag="cc1s")
        half_n = float(EPAD) / 2.0
        nc.vector.tensor_reduce(cc1_sb[:, 0:1], sumA[:], op=AL.add,
                                axis=mybir.AxisListType.X)
        tmp1 = st_p.tile([C, 1], F32, tag="tmp1")
        nc.vector.tensor_scalar(tmp1[:], mv1[:, 0:1], half_n, None, AL.mult)
        nc.vector.tensor_tensor(cc1_sb[:, 0:1], cc1_sb[:, 0:1], tmp1[:], op=AL.add)
        nc.vector.tensor_reduce(cc1_sb[:, 1:2], ssqA[:], op=AL.add,
                                axis=mybir.AxisListType.X)
        nc.vector.tensor_tensor(tmp1[:], mv1[:, 0:1], mv1[:, 0:1], op=AL.mult)
        nc.vector.tensor_tensor(tmp1[:], tmp1[:], mv1[:, 1:2], op=AL.add)
        nc.vector.tensor_scalar(tmp1[:], tmp1[:], half_n, None, AL.mult)
        nc.vector.tensor_tensor(cc1_sb[:, 1:2], cc1_sb[:, 1:2], tmp1[:], op=AL.add)
        nc.sync.dma_start(cc1i_d[:], cc1_sb[:])
        nc.gpsimd.collective_compute(
            "AllReduce", AL.add, replica_groups=rg, ins=[cc1i_d[:]], outs=[cc1o_d[:]]
        )
        cc1r = st_p.tile([C, 2], F32, tag="cc1r")
        nc.sync.dma_start(cc1r[:], cc1o_d[:])
        inv_e = 1.0 / float(N_EDGES)
        mu1 = st_p.tile([C, 1], F32, tag="mu1")
        nc.vector.tensor_scalar(mu1[:], cc1r[:, 0:1], inv_e, None, AL.mult)
        var1 = st_p.tile([C, 1], F32, tag="var1")
        nc.vector.tensor_tensor(var1[:], mu1[:], mu1[:], op=AL.mult)
        nc.vector.tensor_scalar(var1[:], var1[:], -1.0, None, AL.mult)
        ex1 = st_p.tile([C, 1], F32, tag="ex1")
        nc.vector.tensor_scalar(ex1[:], cc1r[:, 1:2], inv_e, None, AL.mult)
        nc.vector.tensor_tensor(var1[:], var1[:], ex1[:], op=AL.add)
        a1, bb1, bb1p1 = _bn_coeffs(nc, st_p, var1, mu1, g1_sb, be1_sb)

        # ---------------- pass 2: BN1+ELU, transpose, scatter ---------------
        # S = elu(T')+1 = where(T'>0, T'+1, exp(T')), with T' = a1*h+bb1
        # folded into the ACT per-partition scale/bias; then one PE transpose
        # per 128-edge block and a one-hot scatter matmul.
        aggT = const.tile([C, NS], BF16, tag="aggT")
        pw = {}
        ident_bf = const.tile([128, 128], BF16, tag="identbf")
        nc.vector.tensor_copy(ident_bf[:], ident[:])
        W2 = 2 * TILE
        for tp_i in range(T // 2):
            h_t = io_p.tile([C, W2], BF16, tag="h2")
            nc.sync.dma_start(h_t[:], h_d[:, tp_i * W2:(tp_i + 1) * W2])
            oh_t = io_p.tile([BLK, 2 * BPT * BIN], BF16, tag="oh")
            nc.sync.dma_start(oh_t[:], oh_d[tp_i])
            ep = mid_p.tile([C, W2], BF16, tag="ep")
            nc.scalar.activation(ep[:], h_t[:], AF.Exp, bias=bb1[:], scale=a1[:])
            tp1 = mid_p.tile([C, W2], BF16, tag="tp1")
            nc.scalar.activation(tp1[:], h_t[:], AF.Identity,
                                 bias=bb1p1[:], scale=a1[:])
            msk = mid_p.tile([C, W2], mybir.dt.uint8, tag="msk")
            nc.vector.tensor_scalar(msk[:], tp1[:], 1.0, None, AL.is_gt)
            nc.vector.copy_predicated(ep[:], msk[:], tp1[:])   # ep := S
            pS = ps_s.tile([128, W2], BF16, tag="pS")
            for bi in range(2 * BPT):
                bsl = slice(bi * BLK, (bi + 1) * BLK)
                nc.tensor.matmul(pS[:, bsl], ep[:, bsl], ident_bf[:],
                                 is_transpose=True, start=True, stop=True)
            s_sb = mid_p.tile([128, W2], BF16, tag="s")
            nc.vector.tensor_copy(s_sb[:], pS[:])
            for bi in range(2 * BPT):
                b = tp_i * 2 * BPT + bi
                q = blk_bin[b]
                w = q // 8
                off = BIN * (q % 8)
                if b == win_first[w]:
                    pw[w] = ps_w.tile([C, WIN], F32, tag="pw", name=f"pw{w}")
                nc.tensor.matmul(
                    pw[w][:, off:off + BIN],
                    s_sb[:, bi * BLK:(bi + 1) * BLK],
                    oh_t[:, bi * BIN:(bi + 1) * BIN],
                    start=(b == bin_first[q]),
                    stop=(b == bin_last[q]),
                )
                if b == win_last[w]:
                    wb = w * WIN
                    ww = min(WIN, NS - wb)
                    nc.vector.tensor_tensor(
                        aggT[:, wb:wb + ww],
                        pw[w][:, :ww],
                        invc_sb[:, wb:wb + ww],
                        op=AL.mult,
                    )

        # ---------------- node MLP ------------------------------------------
        xT_sb = const.tile([C, NS], BF16, tag="xTs")
        nc.sync.dma_start(xT_sb[:], xTs_d[:])
        o2_sb = const.tile([C, NS], BF16, tag="o2")
        for w in range(NWIN):
            wb = w * WIN
            ww = min(WIN, NS - wb)
            p2 = ps_h.tile([C, TILE], F32, tag="ph")
            nc.tensor.matmul(p2[:, :ww], w2x_sb[:], xT_sb[:, wb:wb + ww],
                             start=True, stop=False)
            nc.tensor.matmul(p2[:, :ww], w2a_sb[:], aggT[:, wb:wb + ww],
                             start=False, stop=True)
            nc.scalar.copy(o2_sb[:, wb:wb + ww], p2[:, :ww])

        stats2 = st_p.tile([C, 6 * (NS // ST2_CH)], F32, tag="stats2")
        for k in range(NS // ST2_CH):
            nc.vector.bn_stats(stats2[:, 6 * k:6 * k + 6],
                               o2_sb[:, k * ST2_CH:(k + 1) * ST2_CH])
        mv2 = st_p.tile([C, 2], F32, tag="mv2")
        nc.vector.bn_aggr(mv2[:], stats2[:])
        cc2_sb = st_p.tile([C, 2], F32, tag="cc2s")
        nc.vector.tensor_copy(cc2_sb[:, 0:1], mv2[:, 0:1])
        nc.vector.tensor_tensor(cc2_sb[:, 1:2], mv2[:, 0:1], mv2[:, 0:1], op=AL.mult)
        nc.vector.tensor_tensor(cc2_sb[:, 1:2], cc2_sb[:, 1:2], mv2[:, 1:2], op=AL.add)
        nc.sync.dma_start(cc2i_d[:], cc2_sb[:])
        nc.gpsimd.collective_compute(
            "AllReduce", AL.add, replica_groups=rg, ins=[cc2i_d[:]], outs=[cc2o_d[:]]
        )
        cc2r = st_p.tile([C, 2], F32, tag="cc2r")
        nc.sync.dma_start(cc2r[:], cc2o_d[:])
        mu2 = st_p.tile([C, 1], F32, tag="mu2")
        nc.vector.tensor_scalar(mu2[:], cc2r[:, 0:1], 1.0 / NCORES, None, AL.mult)
        var2 = st_p.tile([C, 1], F32, tag="var2")
        nc.vector.tensor_tensor(var2[:], mu2[:], mu2[:], op=AL.mult)
        nc.vector.tensor_scalar(var2[:], var2[:], -1.0, None, AL.mult)
        ex2 = st_p.tile([C, 1], F32, tag="ex2")
        nc.vector.tensor_scalar(ex2[:], cc2r[:, 1:2], 1.0 / NCORES, None, AL.mult)
        nc.vector.tensor_tensor(var2[:], var2[:], ex2[:], op=AL.add)
        a2, bb2, bb2p1 = _bn_coeffs(nc, st_p, var2, mu2, g2_sb, be2_sb)

        # ---------------- final elementwise + transpose + store -------------
        ep2 = const.tile([C, NS], BF16, tag="ep2")
        nc.scalar.activation(ep2[:], o2_sb[:], AF.Exp, bias=bb2[:], scale=a2[:])
        tq2 = const.tile([C, NS], BF16, tag="tq2")
        nc.scalar.activation(tq2[:], o2_sb[:], AF.Identity,
                             bias=bb2p1[:], scale=a2[:])
        mk2 = const.tile([C, NS], mybir.dt.uint8, tag="mk2")
        nc.vector.tensor_scalar(mk2[:], tq2[:], 1.0, None, AL.is_gt)
        nc.vector.copy_predicated(ep2[:], mk2[:], tq2[:])   # ep2 := elu+1
        nblocks = (NS + BLK - 1) // BLK
        for nb in range(nblocks):
            ob = nb * BLK
            ow = min(BLK, NS - ob)
            pO = ps_s.tile([128, W2], BF16, tag="pS")
            nc.tensor.matmul(pO[:ow, :C], ep2[:, ob:ob + ow], ident_bf[:],
                             is_transpose=True, start=True, stop=True)
            o_sb = mid_p.tile([128, C], F32, tag="osb")
            nc.vector.tensor_scalar(o_sb[:ow, :], pO[:ow, :C], 1.0, -1.0,
                                    AL.mult, AL.add)
            nc.sync.dma_start(out_d[ob:ob + ow, :], o_sb[:ow, :])

    nc.finalize()
    return nc


# --------------------------------------------------------------------------
# entry point
# --------------------------------------------------------------------------

def kernel(x, edge_index, edge_attr, u, batch,
           w1, b1, bn1_w, bn1_b, w2, b2, bn2_w, bn2_b,
           _trace=False):
    in_maps, meta = _host_prep(
        x, edge_index, edge_attr, w1, w2, bn1_w, bn1_b, bn2_w, bn2_b
    )
    key = meta["NB_q"]
    if key not in _prog_cache:
        _prog_cache[key] = _build_program(key)
    nc = _prog_cache[key]
    res = run_bass_kernel_spmd(nc, in_maps, list(range(NCORES)), trace=_trace)
    out = np.concatenate([r["out"] for r in res.results], axis=0)
    if _trace:
        kernel.last_results = res
    return out.astype(np.float32)
